# revision 12
# baseline (speedup 1.0000x reference)
"""GraphConv GNN (5-layer) + global_add_pool + MLP head on 8 Trainium2 cores.

Strategy:
  - Nodes sharded contiguously across 8 cores (12500/core, padded to 12544).
  - Edges assigned to the core owning their dst node; per core, edges are
    grouped by (dst-block of 128 nodes, src table segment of 32768 rows) and
    chunked into runs of 128 edges.
  - Message passing per chunk: dma_gather fetches p[src] rows (fp32, 256B
    table pitch, int16 segment-local indices); DVE multiplies by edge weight
    (cast fp16); a one-hot fp16 S matrix built on-chip (iota == dst_local)
    scatters messages into PSUM via TensorE matmul: aggT[32,128] += msg^T @ S.
  - h update in feature-major: hT = relu(aggT + rT + brel) on DVE+ACT.
  - Linearity trick: segment_sum(x) @ W == segment_sum(x @ W), so features are
    projected to 32 dims BEFORE message passing (p = h @ Wrel), cutting
    gather traffic 4x on layer 1.
  - p shards AllGather'd into a shared table between layers.
  - Pooling via per-block one-hot matmul into [32, GW] PSUM; scattered to a
    [1024, 32] bounce by graph id; AllReduce; MLP head + log_softmax on all
    cores redundantly (core 0's output is returned).
"""
import numpy as np

_CACHE = {}


class Cfg:
    def __init__(self, N=100000, E=1600000, G=1000, F=128, DIM=32, NC=8):
        self.N, self.E, self.G, self.F, self.DIM, self.NC = N, E, G, F, DIM, NC
        assert N % NC == 0
        self.NPC = N // NC
        self.B = -(-self.NPC // 128)          # blocks per core
        self.NPAD = self.B * 128
        self.TROWS = NC * self.NPAD           # table rows (global, padded)
        self.SEG = 32768
        self.NSEG = -(-self.TROWS // self.SEG)
        self.GRP = 7                          # blocks per gather group
        self.NGRP = -(-self.B // self.GRP)
        self.CALL_CHUNKS = 8                  # max 128-edge chunks per dma_gather
        self.HC = -(-G // 128)                # head chunks
        self.GPAD = self.HC * 128


def preprocess(cfg, x, edge_index, batch, edge_weight):
    """Build per-core padded/chunked edge data. Returns dict of host arrays."""
    c = cfg
    src = np.asarray(edge_index[0], dtype=np.int64)
    dst = np.asarray(edge_index[1], dtype=np.int64)
    ew = np.asarray(edge_weight, dtype=np.float32)
    batch = np.asarray(batch, dtype=np.int64)

    # global node -> padded table row
    trow_of = (src // c.NPC) * c.NPAD + (src % c.NPC)

    cnt = np.zeros((c.NC, c.B, c.NSEG), dtype=np.int64)
    percore = []
    for k in range(c.NC):
        base = k * c.NPC
        m = (dst >= base) & (dst < base + c.NPC)
        es_row = trow_of[m]
        ed = (dst[m] - base).astype(np.int32)
        ee = ew[m]
        seg = (es_row // c.SEG).astype(np.int32)
        blk = ed >> 7
        dloc = (ed & 127).astype(np.int32)
        order = np.lexsort((seg, blk))
        es_row, ee, seg, blk, dloc = (a[order] for a in (es_row, ee, seg, blk, dloc))
        np.add.at(cnt[k], (blk, seg), 1)
        percore.append((es_row, ee, seg, blk, dloc))

    # uniform chunk capacities across cores
    C = np.max(-(-cnt // 128), axis=0)        # [B, NSEG] chunks per (block, seg)

    # Gather-order chunks: for grp: for s: for b in grp: C[b,s] chunks.
    # call_list entries: (grp, s, n_chunks, first_gchunk)
    g_bs = []                                  # (b, s) per gather-order chunk
    call_list = []
    call_of_chunk = []
    col_of_chunk = []
    for g in range(c.NGRP):
        blocks = range(g * c.GRP, min((g + 1) * c.GRP, c.B))
        for s in range(c.NSEG):
            off = len(g_bs)
            for b in blocks:
                for _ in range(C[b, s]):
                    g_bs.append((b, s))
            n = len(g_bs) - off
            # split into calls of at most CALL_CHUNKS chunks
            done = 0
            while done < n:
                nn = min(c.CALL_CHUNKS, n - done)
                cid = len(call_list)
                call_list.append((g, s, nn, off + done))
                for i in range(nn):
                    call_of_chunk.append(cid)
                    col_of_chunk.append(i)
                done += nn
    CT = len(g_bs)
    first_gchunk = -np.ones((c.B, c.NSEG), dtype=np.int64)
    for ci, (b, s) in enumerate(g_bs):
        if first_gchunk[b, s] < 0:
            first_gchunk[b, s] = ci

    # Block-major order for S / matmul: for b: for s: chunks.
    # bm_of_gchunk: gather-chunk index -> block-major column index
    bm_order = sorted(range(CT), key=lambda ci: (g_bs[ci][0], ci))
    bm_of_gchunk = np.zeros(CT, dtype=np.int64)
    for col, ci in enumerate(bm_order):
        bm_of_gchunk[ci] = col
    # per block: list of (call_id, col_in_call) in block-major order
    blocks_chunks = [[] for _ in range(c.B)]
    for col, ci in enumerate(bm_order):
        b, s = g_bs[ci]
        blocks_chunks[b].append((call_of_chunk[ci], col_of_chunk[ci]))
    CBMAX = max((len(v) for v in blocks_chunks), default=1)
    CMAX = max((n for (_, _, n, _) in call_list), default=1)
    MAXCPG = max((sum(1 for (g2, _, _, _) in call_list if g2 == g)
                  for g in range(c.NGRP)), default=1)

    idx_wrapped, ew_all, dloc_all = [], [], []
    for k in range(c.NC):
        es_row, ee, seg, blk, dloc = percore[k]
        key = blk.astype(np.int64) * c.NSEG + seg
        changes = np.flatnonzero(np.diff(key)) + 1
        starts = np.concatenate(([0], changes))
        grp_id = np.zeros(len(key), dtype=np.int64)
        grp_id[changes] = 1
        grp_id = np.cumsum(grp_id)
        within = np.arange(len(key)) - starts[grp_id]
        e_chunk = first_gchunk[blk, seg] + (within >> 7)
        e_lane = within & 127
        p = e_chunk * 128 + e_lane
        idx_flat = np.zeros(CT * 128, dtype=np.int16)   # seg-local row 0 pad
        ew_flat = np.zeros(CT * 128, dtype=np.float32)
        idx_flat[p] = (es_row - seg.astype(np.int64) * c.SEG).astype(np.int16)
        ew_flat[p] = ee
        # dloc in block-major columns; padding lanes point at node 0 w/ ew 0
        dl_flat = np.zeros(CT * 128, dtype=np.float16)
        pbm = bm_of_gchunk[e_chunk] * 128 + e_lane
        dl_flat[pbm] = dloc.astype(np.float16)
        ew_all.append(ew_flat.reshape(CT, 128).T.copy())       # [128, CT]
        dloc_all.append(dl_flat.reshape(CT, 128).T.copy())     # [128, CT] block-major
        cols = []
        for (g, s, n, off) in call_list:
            ni = n * 128
            a = idx_flat[off * 128:(off + n) * 128]
            cols.append(a.reshape(ni // 16, 16).T)             # [16, ni/16]
        w16 = np.concatenate(cols, axis=1)
        idx_wrapped.append(np.tile(w16, (8, 1)))               # [128, NI16]

    # block-major chunk column offsets per block
    bm_off = np.zeros(c.B + 1, dtype=np.int64)
    for b in range(c.B):
        bm_off[b + 1] = bm_off[b] + len(blocks_chunks[b])

    # pooling data
    gmin = np.zeros(c.NC, dtype=np.int64)
    gcore = np.zeros(c.NC, dtype=np.int64)
    for k in range(c.NC):
        bk = batch[k * c.NPC:(k + 1) * c.NPC]
        gmin[k] = bk[0]
        gcore[k] = bk[-1] - bk[0] + 1
    GW = int(gcore.max())
    GW = min(-(-GW // 4) * 4, 512)
    assert GW <= 512, f"graphs-per-core window {GW} exceeds 512"
    GSC = -(-GW // 128)
    batchloc = np.full((c.NC, c.B * 128), -1.0, dtype=np.float32)
    gids = np.full((c.NC, 128, GSC), 1 << 20, dtype=np.int32)
    for k in range(c.NC):
        bk = batch[k * c.NPC:(k + 1) * c.NPC]
        batchloc[k, :c.NPC] = (bk - gmin[k]).astype(np.float32)
        n = int(gcore[k])
        ids = np.arange(n, dtype=np.int32) + int(gmin[k])
        full = np.full(GSC * 128, 1 << 20, dtype=np.int32)
        full[:n] = ids
        gids[k] = full.reshape(GSC, 128).T
    xs = []
    for k in range(c.NC):
        xk = np.zeros((c.NPAD, c.F), dtype=np.float32)
        xk[:c.NPC] = x[k * c.NPC:(k + 1) * c.NPC]
        xs.append(xk)

    return dict(
        call_list=call_list, CT=CT, CMAX=CMAX, CBMAX=CBMAX, MAXCPG=MAXCPG,
        blocks_chunks=blocks_chunks, bm_off=bm_off,
        idx=idx_wrapped, ew=ew_all, dloc=dloc_all, xs=xs,
        batchloc=batchloc.reshape(c.NC, c.B, 128), gids=gids, GW=GW, GSC=GSC,
    )


def build_kernel(cfg, pre):
    import concourse.bass as bass
    import concourse.bacc as bacc
    import concourse.tile as tile
    import concourse.mybir as mybir
    from concourse.masks import make_identity

    c = cfg
    dt = mybir.dt
    call_list, CT = pre["call_list"], pre["CT"]
    CMAX, CBMAX, MAXCPG = pre["CMAX"], pre["CBMAX"], pre["MAXCPG"]
    blocks_chunks, bm_off = pre["blocks_chunks"], pre["bm_off"]
    GW, GSC = pre["GW"], pre["GSC"]
    NI16 = sum(n * 8 for (_, _, n, _) in call_list)
    NL = 5
    AF = mybir.ActivationFunctionType
    AL = mybir.AluOpType

    nc = bacc.Bacc("TRN2", target_bir_lowering=False, debug=False,
                   num_devices=c.NC, dynamic_dma_scratch_size=65536)
    rg = [list(range(c.NC))]

    # ---- I/O ----
    x_in = nc.dram_tensor("x", [c.NPAD, c.F], dt.float32, kind="ExternalInput")
    idx_in = nc.dram_tensor("eidx", [128, NI16], dt.int16, kind="ExternalInput")
    ew_in = nc.dram_tensor("eew", [128, CT], dt.float32, kind="ExternalInput")
    dl_in = nc.dram_tensor("edloc", [128, CT], dt.float16, kind="ExternalInput")
    bl_in = nc.dram_tensor("batchloc", [c.B, 128], dt.float32, kind="ExternalInput")
    gid_in = nc.dram_tensor("gids", [128, GSC], dt.int32, kind="ExternalInput")
    wn = {}
    for i in range(1, 6):
        fi = c.F if i == 1 else c.DIM
        for nm, sh in ((f"Wrel{i}", [fi, c.DIM]), (f"Wroot{i}", [fi, c.DIM]),
                       (f"brel{i}", [c.DIM, 1])):
            wn[nm] = nc.dram_tensor(nm, sh, dt.float32, kind="ExternalInput")
    for nm, sh in (("Wlin1", [c.DIM, c.DIM]), ("blin1", [1, c.DIM]),
                   ("Wlin2", [c.DIM, 10]), ("blin2", [1, 10])):
        wn[nm] = nc.dram_tensor(nm, sh, dt.float32, kind="ExternalInput")
    out_t = nc.dram_tensor("out", [c.G, 10], dt.float32, kind="ExternalOutput")

    # ---- internal DRAM ----
    shards, tables = [], []
    for l in range(NL):
        shards.append(nc.dram_tensor(f"pshard{l}", [c.NPAD, 64], dt.float32, kind="Internal"))
        tables.append(nc.dram_tensor(f"table{l}", [c.TROWS, 64], dt.float32,
                                     kind="Internal", addr_space="Shared"))
    rTd = [nc.dram_tensor(f"rTd{i}", [c.DIM, c.NPAD], dt.float32, kind="Internal")
           for i in range(2)]
    g_local = nc.dram_tensor("g_local", [c.GPAD, c.DIM], dt.float32, kind="Internal")
    g_red = nc.dram_tensor("g_red", [c.GPAD, c.DIM], dt.float32,
                           kind="Internal", addr_space="Shared")

    with tile.TileContext(nc) as tc:
        with (
            tc.tile_pool(name="const", bufs=1) as constp,
            tc.tile_pool(name="edge", bufs=1) as edgep,
            tc.tile_pool(name="gbuf", bufs=3) as gbufp,
            tc.tile_pool(name="msgp", bufs=2 * MAXCPG) as msgp,
            tc.tile_pool(name="Sp", bufs=3) as Sp,
            tc.tile_pool(name="sbuf", bufs=3) as sb,
            tc.tile_pool(name="blk", bufs=4) as blkp,
            tc.tile_pool(name="ps", bufs=4, space="PSUM") as psp,
            tc.tile_pool(name="psagg", bufs=3, space="PSUM") as psaggp,
            tc.tile_pool(name="pspool", bufs=1, space="PSUM") as pspoolp,
        ):
            # ---- constants ----
            ident = constp.tile([128, 128], dt.float32)
            make_identity(nc, ident[:])
            iotaS = constp.tile([128, CBMAX * 128], dt.float16)
            nc.gpsimd.iota(iotaS[:].rearrange("p (c e) -> p c e", e=128),
                           pattern=[[0, CBMAX], [1, 128]], base=0,
                           channel_multiplier=0,
                           allow_small_or_imprecise_dtypes=True)
            iotagw = constp.tile([128, GW], dt.float32)
            nc.gpsimd.iota(iotagw[:], pattern=[[1, GW]], base=0,
                           channel_multiplier=0,
                           allow_small_or_imprecise_dtypes=True)
            ones1 = constp.tile([1, 128], dt.float32)
            nc.vector.memset(ones1[:], 1.0)

            idx_sb = edgep.tile([128, NI16], dt.int16)
            nc.sync.dma_start(idx_sb[:], idx_in[:])
            ew_sb = edgep.tile([128, CT], dt.float32)
            nc.sync.dma_start(ew_sb[:], ew_in[:])
            dl_sb = edgep.tile([128, CT], dt.float16)
            nc.sync.dma_start(dl_sb[:], dl_in[:])
            gid_sb = edgep.tile([128, GSC], dt.int32)
            nc.sync.dma_start(gid_sb[:], gid_in[:])

            w_sb = {}
            for nm, t_in in wn.items():
                t = constp.tile(list(t_in.shape), dt.float32, tag=nm, name=f"w_{nm}")
                nc.sync.dma_start(t[:], t_in.ap())
                w_sb[nm] = t

            # ---- layer 1 projections from x ----
            for b in range(c.B):
                xb = blkp.tile([128, c.F], dt.float32, tag="xb")
                nc.sync.dma_start(xb[:], x_in.ap()[b * 128:(b + 1) * 128, :])
                xT_ps = psp.tile([c.F, 128], dt.float32, tag="mm", name=f"xT_ps{b}")
                nc.tensor.transpose(xT_ps[:], xb[:], ident[:])
                xT = blkp.tile([c.F, 128], dt.float32, tag="xTs")
                nc.scalar.activation(xT[:], xT_ps[:], AF.Copy)
                p_ps = psp.tile([128, c.DIM], dt.float32, tag="mm", name=f"p_ps{b}")
                nc.tensor.matmul(p_ps[:], lhsT=xT[:], rhs=w_sb["Wrel1"][:])
                p_sb = blkp.tile([128, c.DIM], dt.float32, tag="psb")
                nc.vector.tensor_copy(p_sb[:], p_ps[:])
                nc.sync.dma_start(shards[0].ap()[b * 128:(b + 1) * 128, 0:c.DIM], p_sb[:])
                r_ps = psp.tile([c.DIM, 128], dt.float32, tag="mm", name=f"r_ps{b}")
                nc.tensor.matmul(r_ps[:], lhsT=w_sb["Wroot1"][:], rhs=xT[:])
                r_sb = blkp.tile([c.DIM, 128], dt.float32, tag="rsb")
                nc.vector.tensor_copy(r_sb[:], r_ps[:])
                nc.sync.dma_start(rTd[0].ap()[:, b * 128:(b + 1) * 128], r_sb[:])

            # ---- layers ----
            gpool_ps = pspoolp.tile([c.DIM, max(GW, 128)], dt.float32, name="gpool")
            pool_done = [0]
            for l in range(NL):
                nc.gpsimd.collective_compute(
                    "AllGather", AL.bypass, replica_groups=rg,
                    ins=[shards[l].ap()], outs=[tables[l].ap()])
                for g in range(c.NGRP):
                    blocks = range(g * c.GRP, min((g + 1) * c.GRP, c.B))
                    # phase 1: gathers + ew-mult per call
                    msg_tiles = {}
                    i16_off = 0
                    for cid, (gg, s, nch, off) in enumerate(call_list):
                        ni = nch * 128
                        if gg != g:
                            i16_off += ni // 16
                            continue
                        gt = gbufp.tile([128, CMAX * 64], dt.float32, tag="g",
                                        name=f"g_l{l}_c{cid}")
                        nc.gpsimd.dma_gather(
                            gt[:, :nch * 64].rearrange("p (c e) -> p c e", e=64),
                            tables[l].ap()[s * c.SEG:min((s + 1) * c.SEG, c.TROWS), :],
                            idx_sb[:, i16_off:i16_off + ni // 16],
                            ni, ni, 64)
                        i16_off += ni // 16
                        msg = msgp.tile([128, CMAX * 32], dt.float16, tag="msg",
                                        name=f"msg_l{l}_c{cid}")
                        nc.vector.tensor_tensor(
                            out=msg[:, :nch * 32].rearrange("p (c e) -> p c e", e=32),
                            in0=gt[:, :nch * 64].rearrange("p (c e) -> p c e", e=64)[:, :, 0:c.DIM],
                            in1=ew_sb[:, off:off + nch].to_broadcast([128, nch, c.DIM]),
                            op=AL.mult)
                        msg_tiles[cid] = msg
                    # phase 2: per block S-build + matmul-scatter + h update
                    for b in blocks:
                        nch_b = len(blocks_chunks[b])
                        if nch_b:
                            S = Sp.tile([128, CBMAX * 128], dt.float16, tag="S",
                                        name=f"S_l{l}_b{b}")
                            nc.vector.tensor_tensor(
                                out=S[:, :nch_b * 128],
                                in0=iotaS[:, :nch_b * 128],
                                in1=dl_sb[:, bm_off[b]:bm_off[b] + nch_b]
                                    .to_broadcast([128, nch_b, 128]),
                                op=AL.is_equal)
                            agg = psaggp.tile([c.DIM, 128], dt.float32, tag="agg",
                                              name=f"agg_l{l}_b{b}")
                            for i, (cid, col) in enumerate(blocks_chunks[b]):
                                msg = msg_tiles[cid]
                                nc.tensor.matmul(
                                    agg[:],
                                    lhsT=msg[:, col * 32:(col + 1) * 32],
                                    rhs=S[:, i * 128:(i + 1) * 128],
                                    start=(i == 0), stop=(i == nch_b - 1))
                        rtb = blkp.tile([c.DIM, 128], dt.float32, tag="rtb")
                        nc.sync.dma_start(rtb[:], rTd[l % 2].ap()[:, b * 128:(b + 1) * 128])
                        hT = blkp.tile([c.DIM, 128], dt.float32, tag="hT")
                        if nch_b:
                            nc.vector.tensor_add(hT[:], agg[:], rtb[:])
                            nc.scalar.activation(hT[:], hT[:], AF.Relu,
                                                 bias=w_sb[f"brel{l + 1}"][:])
                        else:
                            nc.scalar.activation(hT[:], rtb[:],
                                                 AF.Relu, bias=w_sb[f"brel{l + 1}"][:])
                        if l < NL - 1:
                            r_ps = psp.tile([c.DIM, 128], dt.float32, tag="mm",
                                            name=f"rn_ps_l{l}_b{b}")
                            nc.tensor.matmul(r_ps[:], lhsT=w_sb[f"Wroot{l + 2}"][:], rhs=hT[:])
                            r_sb = blkp.tile([c.DIM, 128], dt.float32, tag="rsb")
                            nc.vector.tensor_copy(r_sb[:], r_ps[:])
                            nc.sync.dma_start(rTd[(l + 1) % 2].ap()[:, b * 128:(b + 1) * 128],
                                              r_sb[:])
                            p_ps = psp.tile([128, c.DIM], dt.float32, tag="mm",
                                            name=f"pn_ps_l{l}_b{b}")
                            nc.tensor.matmul(p_ps[:], lhsT=hT[:], rhs=w_sb[f"Wrel{l + 2}"][:])
                            p_sb = blkp.tile([128, c.DIM], dt.float32, tag="psb")
                            nc.vector.tensor_copy(p_sb[:], p_ps[:])
                            nc.sync.dma_start(
                                shards[l + 1].ap()[b * 128:(b + 1) * 128, 0:c.DIM], p_sb[:])
                        else:
                            h_ps = psp.tile([128, c.DIM], dt.float32, tag="mm",
                                            name=f"hn_ps_b{b}")
                            nc.tensor.transpose(h_ps[:], hT[:], ident[:c.DIM, :c.DIM])
                            h_sb = blkp.tile([128, c.DIM], dt.float32, tag="hnodesb")
                            nc.vector.tensor_copy(h_sb[:], h_ps[:])
                            blb = blkp.tile([128, 1], dt.float32, tag="blb")
                            nc.sync.dma_start(blb[:], bl_in.ap()[b:b + 1, :]
                                              .rearrange("a p -> p a"))
                            oh = blkp.tile([128, GW], dt.float32, tag="oh")
                            nc.vector.tensor_tensor(out=oh[:], in0=iotagw[:],
                                                    in1=blb[:].to_broadcast([128, GW]),
                                                    op=AL.is_equal)
                            nc.tensor.matmul(gpool_ps[:, :GW], lhsT=h_sb[:], rhs=oh[:],
                                             start=(pool_done[0] == 0),
                                             stop=(pool_done[0] == c.B - 1))
                            pool_done[0] += 1

            # ---- scatter pooled sums to g_local by graph id ----
            zt = sb.tile([128, c.GPAD * c.DIM // 128], dt.float32, tag="zero")
            nc.vector.memset(zt[:], 0.0)
            nc.sync.dma_start(
                g_local.ap().rearrange("(p q) f -> p (q f)", p=128), zt[:])
            gpool_sb = sb.tile([c.DIM, GW], dt.float32, tag="gpool_sb")
            nc.vector.tensor_copy(gpool_sb[:], gpool_ps[:, :GW])
            for j in range(GSC):
                gwn = min(128, GW - j * 128)
                gsl_ps = psp.tile([128, c.DIM], dt.float32, tag="mm", name=f"gsl_ps{j}")
                nc.tensor.transpose(gsl_ps[:gwn, :], gpool_sb[:, j * 128:j * 128 + gwn],
                                    ident[:c.DIM, :c.DIM])
                gsl = blkp.tile([128, c.DIM], dt.float32, tag="gsl")
                nc.vector.tensor_copy(gsl[:gwn, :], gsl_ps[:gwn, :])
                nc.gpsimd.indirect_dma_start(
                    out=g_local.ap(), in_=gsl[:gwn, :],
                    out_offset=bass.IndirectOffsetOnAxis(ap=gid_sb[:gwn, j:j + 1], axis=0),
                    in_offset=None,
                    bounds_check=c.GPAD - 1, oob_is_err=False)
            nc.gpsimd.collective_compute(
                "AllReduce", AL.add, replica_groups=rg,
                ins=[g_local.ap()], outs=[g_red.ap()])

            # ---- head ----
            for hc in range(c.HC):
                rows = min(128, c.G - hc * 128)
                gch = blkp.tile([128, c.DIM], dt.float32, tag="gch")
                nc.sync.dma_start(gch[:], g_red.ap()[hc * 128:(hc + 1) * 128, :])
                gT_ps = psp.tile([c.DIM, 128], dt.float32, tag="mm", name=f"gT_ps{hc}")
                nc.tensor.transpose(gT_ps[:], gch[:], ident[:])
                gT = blkp.tile([c.DIM, 128], dt.float32, tag="gTs")
                nc.vector.tensor_copy(gT[:], gT_ps[:])
                z1_ps = psp.tile([c.DIM, 128], dt.float32, tag="mm", name=f"z1_ps{hc}")
                nc.tensor.matmul(z1_ps[:], lhsT=w_sb["Wlin1"][:], rhs=gT[:],
                                 start=True, stop=False)
                nc.tensor.matmul(z1_ps[:], lhsT=w_sb["blin1"][:],
                                 rhs=ones1[:], start=False, stop=True)
                z1 = blkp.tile([c.DIM, 128], dt.float32, tag="z1s")
                nc.scalar.activation(z1[:], z1_ps[:], AF.Relu)
                z2_ps = psp.tile([128, 16], dt.float32, tag="mm", name=f"z2_ps{hc}")
                nc.tensor.matmul(z2_ps[:, :10], lhsT=z1[:], rhs=w_sb["Wlin2"][:],
                                 start=True, stop=False)
                nc.tensor.matmul(z2_ps[:, :10], lhsT=ones1[:], rhs=w_sb["blin2"][:],
                                 start=False, stop=True)
                negm = blkp.tile([128, 1], dt.float32, tag="negm")
                nc.vector.tensor_reduce(negm[:], z2_ps[:, :10], axis=mybir.AxisListType.X,
                                        op=AL.max, negate=True)
                es = blkp.tile([128, 10], dt.float32, tag="es")
                sumexp = blkp.tile([128, 1], dt.float32, tag="sumexp")
                nc.scalar.activation(es[:], z2_ps[:, :10], AF.Exp,
                                     bias=negm[:], accum_out=sumexp[:])
                lse = blkp.tile([128, 1], dt.float32, tag="lse")
                nc.scalar.activation(lse[:], sumexp[:], AF.Ln)
                oc = blkp.tile([128, 10], dt.float32, tag="oc")
                nc.vector.tensor_scalar(out=oc[:], in0=z2_ps[:, :10], scalar1=negm[:],
                                        scalar2=lse[:], op0=AL.add, op1=AL.subtract)
                nc.sync.dma_start(out_t.ap()[hc * 128:hc * 128 + rows, :], oc[:rows, :])

    nc.compile()
    return nc


def _weights_host(cfg, inputs):
    w = {}
    for i in range(1, 6):
        w[f"Wrel{i}"] = np.asarray(inputs[f"Wrel{i}"], dtype=np.float32)
        w[f"Wroot{i}"] = np.asarray(inputs[f"Wroot{i}"], dtype=np.float32)
        w[f"brel{i}"] = np.asarray(inputs[f"brel{i}"], dtype=np.float32).reshape(-1, 1)
    w["Wlin1"] = np.asarray(inputs["Wlin1"], dtype=np.float32)
    w["blin1"] = np.asarray(inputs["blin1"], dtype=np.float32).reshape(1, -1)
    w["Wlin2"] = np.asarray(inputs["Wlin2"], dtype=np.float32)
    w["blin2"] = np.asarray(inputs["blin2"], dtype=np.float32).reshape(1, -1)
    return w


def _in_maps(cfg, pre, w):
    maps = []
    for k in range(cfg.NC):
        m = dict(
            x=pre["xs"][k], eidx=np.ascontiguousarray(pre["idx"][k]),
            eew=np.ascontiguousarray(pre["ew"][k]),
            edloc=np.ascontiguousarray(pre["dloc"][k]),
            batchloc=np.ascontiguousarray(pre["batchloc"][k]),
            gids=np.ascontiguousarray(pre["gids"][k]),
        )
        m.update(w)
        maps.append(m)
    return maps


def run(cfg, inputs):
    from concourse import bass_utils
    c = cfg
    pre = preprocess(cfg, np.asarray(inputs["x"], dtype=np.float32),
                     np.asarray(inputs["edge_index"]),
                     np.asarray(inputs["batch"]),
                     np.asarray(inputs["edge_weight"], dtype=np.float32))
    key = ("k", c.N, c.E, c.NC, pre["CT"], pre["GW"])
    if key not in _CACHE:
        _CACHE[key] = build_kernel(cfg, pre)
    nc = _CACHE[key]
    res = bass_utils.run_bass_kernel_spmd(
        nc, _in_maps(cfg, pre, _weights_host(cfg, inputs)),
        core_ids=list(range(c.NC)))
    return res.results[0]["out"].astype(np.float32)


def kernel(**inputs):
    cfg = Cfg()
    return run(cfg, inputs)
